# revision 1
# baseline (speedup 1.0000x reference)
"""Trainium2 Bass kernel for nn_GaussianModel (gaussian splatting into a 256^3 volume).

Strategy
--------
Each of N gaussians splats a separable 3D gaussian into a <=26^3 voxel window:
    vol[x,y,z] += I * exp(-0.5*(dx^2+dy^2+dz^2)/sigma^2)   (hard-masked window)

The contribution is separable (w = wx (x) wy (x) wz), so summing over
gaussians is a CONTRACTION over the gaussian axis -> a matmul on the PE:

    vol[y, (x,z)] = sum_g  Wy[g, y] * Wxz[g, (x,z)],   Wxz = Wx (x)row Wz

Sharding: the OUTPUT x-axis is split into 8 slabs of 32 (one per core); each
core processes only gaussians whose x-window intersects its slab (~1.1x
replication).  No collective; the host concatenates slabs.

Per core the volume is 4 quadrants (y-half x z-half).  Gaussians are bucketed
per quadrant (duplicated when straddling).  The bucket loads are heavily
skewed and core-dependent, so buckets are assigned to 4 SLOTS ordered by
size per core (slot block counts = rank-max over cores, ~11 blocks vs 22 for
fixed buckets).  The quadrant a slot belongs to is pure host data: gaussian
params are pre-shifted to quadrant-local coordinates (y_loc = y - 128*hy) and
the kernel writes slot k to a static region of a [4,32,128,128] output that
the HOST unshuffles into the volume.

Per 128-gaussian block:
  - dense axis weights w = mask * exp(-(s'*(i - mu))^2) * I via ACT
    (Square, Exp - one table set) + fused scalar_tensor_tensor masks on DVE
  - row-wise Kronecker Wxz[g, xl*128+z] = Wx[g,xl]*Wz[g,z] via 32
    tensor_scalar muls split across DVE / GPSIMD / ACT
  - 8 matmuls [K=128, M=128, N=512] accumulating in PSUM over blocks
PSUM is evacuated via DVE/ACT copies and DMAed to the slot region.
"""

import sys
import numpy as np

for _p in ("/opt/trn_rl_repo", "/root/.axon_site/_ro/trn_rl_repo"):
    if _p not in sys.path:
        sys.path.append(_p)

SHAPE = (256, 256, 256)
N_CORES = 8
SLAB = SHAPE[0] // N_CORES          # 32 x-planes per core
W = 26                              # reference's fixed window size
NPRM = 12                           # params per gaussian (padded)
NSLOT = 4

# param column indices
C_BY, C_BZ, C_BX, C_SP, C_LNI, C_LOY, C_HIY, C_LOZ, C_HIZ, C_LOX, C_HIX = range(11)


def _host_pack(centers, sigmas, intensities):
    """Replicate the reference's f32 index math exactly; slot/pack per core."""
    f32 = np.float32
    c = np.asarray(centers, f32)
    sg = np.asarray(sigmas, f32)
    it = np.asarray(intensities, f32)
    n = c.shape[0]

    scale = f32(255.0)
    cv = c * scale                                     # [N,3] voxel-space centers
    cut = (f32(3.0) * sg)[:, None] * np.full((3,), scale, f32)[None, :]
    min_i = np.maximum(cv - cut, f32(0.0)).astype(np.int32)
    max_i = np.minimum((np.minimum(cv + cut, scale) + f32(1.0)).astype(np.int32), 256)
    hi = np.minimum(max_i, min_i + W).astype(f32)      # reference clips to W window
    lo = min_i.astype(f32)

    active_cnt = int((sg > 0).sum())
    keep = (np.arange(n) < active_cnt) & (sg > 0)

    sp = np.zeros(n, f32)
    sp[keep] = f32(1.0) / (f32(np.sqrt(2.0)) * scale * sg[keep])
    lni = np.where(it > 0, np.log(np.maximum(it, 1e-38)).astype(f32), f32(-1e30))

    gidx = np.nonzero(keep)[0]
    buckets = []                                       # [core][4] -> gaussian idx
    for i in range(N_CORES):
        x0, x1 = SLAB * i, SLAB * (i + 1)
        in_core = gidx[(lo[gidx, 0] < x1) & (hi[gidx, 0] > x0)]
        bl = []
        for hy in range(2):
            iny = in_core[(lo[in_core, 1] < 128 * (hy + 1)) & (hi[in_core, 1] > 128 * hy)]
            for hz in range(2):
                bl.append(((hy, hz),
                           iny[(lo[iny, 2] < 128 * (hz + 1)) & (hi[iny, 2] > 128 * hz)]))
        # order buckets by size descending -> slots
        bl.sort(key=lambda t: -len(t[1]))
        buckets.append(bl)

    nb = [max(1, max((len(buckets[i][k][1]) + 127) // 128 for i in range(N_CORES)))
          for k in range(NSLOT)]
    nbtot = sum(nb)

    payloads = []
    for i in range(N_CORES):
        prm = np.zeros((nbtot, 128, NPRM), f32)
        slotmap = []
        base = 0
        for k in range(NSLOT):
            (hy, hz), g = buckets[i][k]
            slotmap.append((hy, hz))
            kk = len(g)
            rows = prm[base:base + nb[k]].reshape(-1, NPRM)
            yo, zo = f32(128.0 * hy), f32(128.0 * hz)
            # axis order in the input arrays: 0=x, 1=y, 2=z
            rows[:kk, C_BY] = sp[g] * (yo - cv[g, 1])
            rows[:kk, C_BZ] = sp[g] * (zo - cv[g, 2])
            rows[:kk, C_BX] = sp[g] * (0.0 - cv[g, 0])
            rows[:kk, C_SP] = sp[g]
            rows[:kk, C_LNI] = lni[g]
            rows[:kk, C_LOY] = lo[g, 1] - yo
            rows[:kk, C_HIY] = hi[g, 1] - yo
            rows[:kk, C_LOZ] = lo[g, 2] - zo
            rows[:kk, C_HIZ] = hi[g, 2] - zo
            rows[:kk, C_LOX] = lo[g, 0]
            rows[:kk, C_HIX] = hi[g, 0]
            base += nb[k]
        iotax = np.broadcast_to(
            (SLAB * i + np.arange(SLAB)).astype(f32), (128, SLAB)).copy()
        payloads.append({"prm": prm, "iotax": iotax, "slotmap": slotmap})

    return payloads, nb


def _build_kernel(nb, reps=1):
    """Build + compile the 8-core SPMD Bass program for slot block counts nb.

    reps>1 repeats the whole compute (identical work/results) for benchmarking:
    steady-state HW time = (t(R) - t(1)) / (R - 1).
    """
    from concourse import bacc, tile
    import concourse.bass as bass
    import concourse.mybir as mybir

    f32 = mybir.dt.float32
    AF = mybir.ActivationFunctionType
    OP = mybir.AluOpType

    nbtot = sum(nb)

    nc = bacc.Bacc("TRN2", target_bir_lowering=False, debug=False,
                   num_devices=N_CORES)
    io_t = nc.dram_tensor("iota", (128, 128), f32, kind="ExternalInput")
    iox_t = nc.dram_tensor("iotax", (128, SLAB), f32, kind="ExternalInput")
    prm_t = nc.dram_tensor("prm", (nbtot, 128, NPRM), f32, kind="ExternalInput")
    vol_t = nc.dram_tensor("vol", (NSLOT, SLAB, 128, 128), f32, kind="ExternalOutput")

    with tile.TileContext(nc) as tc:
        with (
            tc.tile_pool(name="const", bufs=1) as cpool,
            tc.tile_pool(name="work", bufs=4) as wpool,
            tc.tile_pool(name="kron", bufs=3) as kpool,
            tc.tile_pool(name="evac", bufs=2) as opool,
            tc.tile_pool(name="psum", bufs=1, space="PSUM") as ppool,
        ):
            iota_sb = cpool.tile([128, 128], f32)
            nc.sync.dma_start(iota_sb[:], io_t.ap())
            iotax_sb = cpool.tile([128, SLAB], f32)
            nc.sync.dma_start(iotax_sb[:], iox_t.ap())
            prm_sb = cpool.tile([128, nbtot * NPRM], f32)
            for blk in range(nbtot):
                nc.sync.dma_start(
                    prm_sb[:, blk * NPRM:(blk + 1) * NPRM], prm_t.ap()[blk])

            def axis_weights(blk, io_ap, width, c_b, c_lo, c_hi, lni_ap):
                """w[g, :width] = mask * exp(-(sp*io + b)^2 + lnI)"""
                P = lambda col: prm_sb[:, blk * NPRM + col: blk * NPRM + col + 1]
                sq = wpool.tile([128, width], f32, tag=f"sq{width}")
                nc.scalar.activation(sq[:], io_ap, AF.Square,
                                     bias=P(c_b), scale=P(C_SP))
                e = wpool.tile([128, width], f32, tag=f"e{width}")
                nc.scalar.activation(e[:], sq[:], AF.Exp,
                                     bias=lni_ap if lni_ap is not None else 0.0,
                                     scale=-1.0)
                t = wpool.tile([128, width], f32, tag=f"t{width}")
                nc.vector.scalar_tensor_tensor(
                    t[:], io_ap, P(c_hi), e[:], op0=OP.is_lt, op1=OP.mult)
                w = wpool.tile([128, width], f32, tag=f"w{width}")
                nc.vector.scalar_tensor_tensor(
                    w[:], io_ap, P(c_lo), t[:], op0=OP.is_ge, op1=OP.mult)
                return w

            for rep in range(reps):
                base = 0
                for k in range(NSLOT):
                    nblk = nb[k]
                    psums = [ppool.tile([128, 512], f32, name=f"ps{i}",
                                        tag=f"ps{i}")
                             for i in range(8)]
                    for j in range(nblk):
                        blk = base + j
                        P = lambda col: prm_sb[:, blk * NPRM + col: blk * NPRM + col + 1]
                        wy = axis_weights(blk, iota_sb[:], 128, C_BY, C_LOY,
                                          C_HIY, P(C_LNI))
                        wz = axis_weights(blk, iota_sb[:], 128, C_BZ, C_LOZ,
                                          C_HIZ, None)
                        wx = axis_weights(blk, iotax_sb[:], SLAB, C_BX, C_LOX,
                                          C_HIX, None)
                        wxz = kpool.tile([128, SLAB * 128], f32, tag="wxz")
                        for xl in range(SLAB):
                            dst = wxz[:, xl * 128:(xl + 1) * 128]
                            sc = wx[:, xl:xl + 1]
                            if xl % 8 in (5, 6, 7):
                                nc.gpsimd.tensor_scalar(dst, wz[:], sc, None, OP.mult)
                            else:
                                nc.vector.tensor_scalar(dst, wz[:], sc, None, OP.mult)
                        for nn in range(8):
                            nc.tensor.matmul(
                                psums[nn][:], wy[:], wxz[:, nn * 512:(nn + 1) * 512],
                                start=(j == 0), stop=(j == nblk - 1))
                    st = opool.tile([128, 8 * 512], f32, tag="st")
                    for nn in range(8):
                        sl = st[:, nn * 512:(nn + 1) * 512]
                        if nn % 2 == 0:
                            nc.vector.tensor_copy(sl, psums[nn][:])
                        else:
                            nc.scalar.copy(sl, psums[nn][:])
                    dst = vol_t.ap()[k].rearrange("x y z -> y x z")
                    nc.sync.dma_start(dst, st[:].rearrange("p (x z) -> p x z", x=SLAB))
                    base += nblk

    nc.compile()
    return nc


def _run(inputs, trace=False):
    from concourse import bass_utils

    payloads, nb = _host_pack(
        inputs["centers"], inputs["sigmas"], inputs["intensities"])
    nc = _build_kernel(nb)

    iota_np = np.broadcast_to(
        np.arange(128, dtype=np.float32), (128, 128)).copy()
    in_maps = [
        {"iota": iota_np, "iotax": p["iotax"], "prm": p["prm"]}
        for p in payloads
    ]
    res = bass_utils.run_bass_kernel_spmd(
        nc, in_maps, core_ids=list(range(N_CORES)), trace=trace)

    out = np.empty(SHAPE, np.float32)
    for i in range(N_CORES):
        v = res.results[i]["vol"]          # [4, 32, 128, 128]
        for k, (hy, hz) in enumerate(payloads[i]["slotmap"]):
            out[SLAB * i:SLAB * (i + 1),
                128 * hy:128 * (hy + 1),
                128 * hz:128 * (hz + 1)] = v[k]
    return out, res


def kernel(centers, sigmas, intensities):
    out, _ = _run({"centers": centers, "sigmas": sigmas,
                   "intensities": intensities})
    return out


if __name__ == "__main__":
    rng = np.random.default_rng(0)
    c = rng.random((100, 3), np.float32)
    s = (0.004 + 0.011 * rng.random(100)).astype(np.float32)
    i = rng.random(100, np.float32)
    v = kernel(centers=c, sigmas=s, intensities=i)
    print(v.shape, v.dtype, v.sum())

